# revision 15
# baseline (speedup 1.0000x reference)
"""Trainium2 Bass kernel for nn_LinkPredictor (2-layer GCN + edge-dot decode).

Strategy (8 NeuronCores, SPMD), v3:
  - Nodes sharded: core c owns rows [c*12544, (c+1)*12544) of the padded
    node table (N=100000 padded to 100352 = 8*98*128).
  - dinv folded into node features: table rows hold hs = dinv[n] * (prev @ W);
    output z = relu(dinv[v]*(agg + hs[v]) + b).
  - Node tables in DRAM are band-major contiguous [4*25088, D]: collective
    chunk K holds rows c*3136+j of each core's shard; 4 pipelined AllGathers
    per layer write slices of one tensor.
  - Gathers use signed-int16 indices with offset bases (hardware sign-extends)
    so only 2 gather-bands cover all 100352 rows -> bigger edge groups,
    ~10% less padding. One dma_gather per (window-batch, gather-band).
  - Aggregation: pure one-hot S (single-op DVE tensor_scalar is_equal, dead
    slots dstloc=-1; a fraction of tiles built on ScalarE via Abs+Relu)
    feeding PE matmul accumulation into PSUM. Self-loop = identity matmul;
    bias = rank-1 matmul (skipped when b==0); relu+dinv scale on ScalarE.
  - xT and hs tiles resident in SBUF as wide [128, 12544] tiles.
  - Next layer's first batch (gather-band A) is prefetched during the
    current layer's tail through a shared M pool.
  - Decode: gathers z[s], z[d] by gather-band pair, DVE mult + ACT accum.
"""
import contextlib
import math
import numpy as np
import ml_dtypes

import concourse.bass as bass
import concourse.tile as tile
from concourse import bacc, mybir
from concourse.bass_utils import run_bass_kernel_spmd
from concourse.tile_rust import add_dep_helper

F32 = mybir.dt.float32
BF16 = mybir.dt.bfloat16
I16 = mybir.dt.int16
BF = ml_dtypes.bfloat16
ACTF = mybir.ActivationFunctionType


class Cfg:
    def __init__(self, N=100000, E=1600000, EL=100000, D=128, ncores=8,
                 nw=98, nbands=4, wb=3, act_tenths=3, prefetch=1):
        self.N, self.E, self.EL, self.D, self.NC = N, E, EL, D, ncores
        self.NW = nw                      # windows (128 nodes each) per core
        self.SHARD = nw * 128             # nodes per core (padded)
        self.NP = self.SHARD * ncores     # padded node count
        assert self.NP >= N
        self.NB = nbands                  # collective chunks per shard
        assert self.SHARD % nbands == 0
        self.BROWS = self.SHARD // nbands # shard rows per collective chunk
        self.TBROWS = self.BROWS * ncores # table rows per chunk (band-major)
        self.NGB = 2                      # gather bands (int16 +- offset)
        self.GBASE = [32768, 98304]       # table row base per gather band
        self.WB = wb                      # windows per gather/aggregate batch
        self.NBATCH = math.ceil(nw / wb)
        self.ACT_TENTHS = act_tenths      # S-build tiles on ScalarE /10
        self.PREFETCH = prefetch          # next-layer batches gathered early


DEFAULT = Cfg()


def _wrap_idxs(idx):
    """[n] ints -> [128, n//16] int16 wrapped in 16 partitions, replicated 8x."""
    n = len(idx)
    assert n % 16 == 0
    w = np.asarray(idx, dtype=np.int16).reshape(n // 16, 16).T
    return np.ascontiguousarray(np.tile(w, (8, 1)))


def host_prep(cfg, x, edge_index, edge_label_index, W1, b1, W2, b2):
    """All host-side sharding/layout. Returns (per-core input maps, meta)."""
    c = cfg
    src = np.asarray(edge_index[0], dtype=np.int64)
    dst = np.asarray(edge_index[1], dtype=np.int64)
    deg = np.bincount(dst, minlength=c.N).astype(np.float64) + 1.0
    dinv = 1.0 / np.sqrt(deg)                      # [N]
    dinv_p = np.ones(c.NP, dtype=np.float64)
    dinv_p[:c.N] = dinv
    dinv_f = dinv_p.astype(np.float32)

    def bmaj_of(n):
        """band-major table row of node id n."""
        cc, r = n // c.SHARD, n % c.SHARD
        K = r // c.BROWS
        return K * c.TBROWS + cc * c.BROWS + (r - K * c.BROWS)

    bmaj_src = bmaj_of(src)
    gband_src = (bmaj_src >= 65536).astype(np.int64)
    core_of = dst // c.SHARD
    w_of = (dst % c.SHARD) // 128
    dloc = dst % 128

    key = (core_of * c.NW + w_of) * c.NGB + gband_src
    ngroups = c.NC * c.NW * c.NGB
    order = np.argsort(key, kind="stable")
    counts = np.bincount(key, minlength=ngroups).reshape(c.NC, c.NW, c.NGB)
    starts = np.zeros(ngroups + 1, dtype=np.int64)
    np.cumsum(np.bincount(key, minlength=ngroups), out=starts[1:])

    T = np.ceil(counts.max(axis=0) / 128).astype(np.int64)     # [NW, NGB]
    TOT_TILES = int(T.sum())
    span_tiles = np.zeros((c.NBATCH, c.NGB), dtype=np.int64)
    for b in range(c.NBATCH):
        wlo, whi = b * c.WB, min((b + 1) * c.WB, c.NW)
        for g in range(c.NGB):
            span_tiles[b, g] = T[wlo:whi, g].sum()
    TOT = TOT_TILES * 128

    idx_arr = np.zeros((c.NC, TOT), dtype=np.int64)
    dloc_arr = np.full((c.NC, TOT), -1.0, dtype=np.float32)
    for core in range(c.NC):
        pos = 0
        for b in range(c.NBATCH):
            wlo, whi = b * c.WB, min((b + 1) * c.WB, c.NW)
            for g in range(c.NGB):
                for w in range(wlo, whi):
                    gk = (core * c.NW + w) * c.NGB + g
                    eids = order[starts[gk]:starts[gk + 1]]
                    n = len(eids)
                    idx_arr[core, pos:pos + n] = bmaj_src[eids] - c.GBASE[g]
                    dloc_arr[core, pos:pos + n] = dloc[eids]
                    pos += int(T[w, g]) * 128
        assert pos == TOT
    assert idx_arr.min() >= -32768 and idx_arr.max() <= 32767

    # decode: label edge j -> core j // ELC; groups by (gband(s), gband(d))
    assert c.EL % c.NC == 0
    ELC = c.EL // c.NC
    ls = np.asarray(edge_label_index[0], dtype=np.int64)
    ld = np.asarray(edge_label_index[1], dtype=np.int64)
    bs, bd = bmaj_of(ls), bmaj_of(ld)
    gs = (bs >= 65536).astype(np.int64)
    gd = (bd >= 65536).astype(np.int64)
    gdec = gs * c.NGB + gd
    NG_DEC = c.NGB * c.NGB
    cnt_dec = np.zeros((c.NC, NG_DEC), dtype=np.int64)
    for core in range(c.NC):
        cnt_dec[core] = np.bincount(gdec[core * ELC:(core + 1) * ELC],
                                    minlength=NG_DEC)
    Tdec = np.ceil(cnt_dec.max(axis=0) / 128).astype(np.int64)   # [NG_DEC]
    gorder = sorted(range(NG_DEC), key=lambda g: (max(g // c.NGB, g % c.NGB), g))
    TOT_DEC = int(Tdec.sum()) * 128
    idx_s = np.zeros((c.NC, TOT_DEC), dtype=np.int64)
    idx_d = np.zeros((c.NC, TOT_DEC), dtype=np.int64)
    slot2j = np.full((c.NC, TOT_DEC), -1, dtype=np.int64)
    for core in range(c.NC):
        jlo = core * ELC
        kk = gdec[jlo:jlo + ELC]
        o = np.argsort(kk, kind="stable")
        st = np.zeros(NG_DEC + 1, dtype=np.int64)
        np.cumsum(np.bincount(kk, minlength=NG_DEC), out=st[1:])
        pos = 0
        for g in gorder:
            js = o[st[g]:st[g + 1]] + jlo
            n = len(js)
            idx_s[core, pos:pos + n] = bs[js] - c.GBASE[g // c.NGB]
            idx_d[core, pos:pos + n] = bd[js] - c.GBASE[g % c.NGB]
            slot2j[core, pos:pos + n] = js
            pos += int(Tdec[g]) * 128
        assert pos == TOT_DEC

    xp = np.zeros((c.NP, c.D), dtype=np.float32)
    xp[:c.N] = np.asarray(x, dtype=np.float32)
    use_b1 = bool(np.any(np.asarray(b1)))
    use_b2 = bool(np.any(np.asarray(b2)))

    in_maps = []
    for core in range(c.NC):
        sl = slice(core * c.SHARD, (core + 1) * c.SHARD)
        dsh = dinv_f[sl]
        m = {
            "xT": np.ascontiguousarray(xp[sl].T).astype(BF),
            "W1": np.asarray(W1, dtype=np.float32).astype(BF),
            "W2": np.asarray(W2, dtype=np.float32).astype(BF),
            "dinv": np.ascontiguousarray(dsh.reshape(c.NW, 128).T),
            "gidx": _wrap_idxs(idx_arr[core]),
            "dstloc": np.ascontiguousarray(
                dloc_arr[core].reshape(TOT_TILES, 128).T),
            "negdst": np.ascontiguousarray(
                (-dloc_arr[core]).reshape(TOT_TILES, 128).T),
            "didx_s": _wrap_idxs(idx_s[core]),
            "didx_d": _wrap_idxs(idx_d[core]),
        }
        if use_b1 or use_b2:
            m["b1r"] = np.asarray(b1, np.float32)[None, :].astype(BF)
            m["b2r"] = np.asarray(b2, np.float32)[None, :].astype(BF)
            m["invd"] = (1.0 / dsh)[None, :].astype(BF)
        in_maps.append(m)
    meta = dict(T=T, span_tiles=span_tiles, TOT=TOT, TOT_TILES=TOT_TILES,
                Tdec=Tdec, gorder=gorder, TOT_DEC=TOT_DEC, slot2j=slot2j,
                use_b1=use_b1, use_b2=use_b2)
    return in_maps, meta


def build_program(cfg, meta, num_cores=None):
    c = cfg
    NCores = num_cores or c.NC
    T, span_tiles = meta["T"], meta["span_tiles"]
    TOT, TOT_TILES = meta["TOT"], meta["TOT_TILES"]
    Tdec, gorder, TOT_DEC = meta["Tdec"], meta["gorder"], meta["TOT_DEC"]
    use_b = {1: meta["use_b1"], 2: meta["use_b2"]}
    D = c.D
    TBMAX = int(span_tiles.sum(axis=1).max())
    # collective chunks each gather band depends on (band-major rows)
    GB_CHUNKS = [[], []]
    for K in range(c.NB):
        lo, hi = K * c.TBROWS, (K + 1) * c.TBROWS
        if lo < 65536:
            GB_CHUNKS[0].append(K)
        if hi > 65536:
            GB_CHUNKS[1].append(K)

    nc = bacc.Bacc("TRN2", target_bir_lowering=False, debug=False,
                   num_devices=NCores, num_swdge_queues=4)
    NQ = 4

    xT_in = nc.dram_tensor("xT", [D, c.SHARD], BF16, kind="ExternalInput")
    W1_in = nc.dram_tensor("W1", [D, D], BF16, kind="ExternalInput")
    W2_in = nc.dram_tensor("W2", [D, D], BF16, kind="ExternalInput")
    dinv_in = nc.dram_tensor("dinv", [128, c.NW], F32, kind="ExternalInput")
    gidx_in = nc.dram_tensor("gidx", [128, TOT // 16], I16, kind="ExternalInput")
    dstloc_in = nc.dram_tensor("dstloc", [128, TOT_TILES], F32, kind="ExternalInput")
    negdst_in = nc.dram_tensor("negdst", [128, TOT_TILES], F32, kind="ExternalInput")
    didx_s_in = nc.dram_tensor("didx_s", [128, TOT_DEC // 16], I16, kind="ExternalInput")
    didx_d_in = nc.dram_tensor("didx_d", [128, TOT_DEC // 16], I16, kind="ExternalInput")
    if use_b[1] or use_b[2]:
        b1_in = nc.dram_tensor("b1r", [1, D], BF16, kind="ExternalInput")
        b2_in = nc.dram_tensor("b2r", [1, D], BF16, kind="ExternalInput")
        invd_in = nc.dram_tensor("invd", [1, c.SHARD], BF16, kind="ExternalInput")
    dots_out = nc.dram_tensor("dots", [128, TOT_DEC // 128], F32, kind="ExternalOutput")

    shard_b = {l: [nc.dram_tensor(f"shard{l}_{k}", [c.BROWS, D], BF16)
                   for k in range(c.NB)] for l in (1, 2, 3)}
    table = {l: nc.dram_tensor(f"table{l}", [c.NB * c.TBROWS, D], BF16,
                               addr_space="Shared") for l in (1, 2, 3)}

    def tslice(l, g):
        base = c.GBASE[g]
        hi = min(base + 32768, c.NB * c.TBROWS)
        return table[l][base:hi, :]

    iota_dram = nc.inline_tensor(
        np.tile(np.arange(128, dtype=np.float32), (128, 1)).astype(BF), "iota_c")
    ident_dram = nc.inline_tensor(np.eye(128, dtype=np.float32).astype(BF), "ident_c")

    core_ids = list(range(NCores))
    gst = {"count": 0, "prev": None}
    ccst = {}                            # (l, K) -> collective instruction

    def emit_gather(out_ap, in_ap, idx_ap, n_idx, deps=()):
        q = gst["count"] % NQ
        inst = nc.gpsimd.dma_gather(out_ap, in_ap, idx_ap, n_idx, n_idx, D,
                                    queue_num=q, single_packet=False)
        if gst["prev"] is not None:
            add_dep_helper(inst.ins, gst["prev"].ins, sync=False,
                           reason="pin swdge queue order")
        for dcc in deps:
            add_dep_helper(inst.ins, dcc.ins, sync=True,
                           reason="gather after collective")
        gst["prev"] = inst
        gst["count"] += 1
        return inst

    def emit_collective(l, K):
        cc = nc.gpsimd.collective_compute(
            "AllGather", mybir.AluOpType.bypass,
            replica_groups=[core_ids],
            ins=[shard_b[l][K][:]],
            outs=[table[l][K * c.TBROWS:(K + 1) * c.TBROWS, :]],
        )
        if gst["prev"] is not None:
            add_dep_helper(cc.ins, gst["prev"].ins, sync=False,
                           reason="order on gpsimd")
        gst["prev"] = cc
        ccst[(l, K)] = cc
        return cc

    def write_window(l, w, src_ap, done_k):
        """DMA window w rows into banded shards; fire collectives when a
        band completes."""
        lo, hi = w * 128, (w + 1) * 128
        k0, k1 = lo // c.BROWS, (hi - 1) // c.BROWS
        for K in range(k0, k1 + 1):
            rlo, rhi = max(lo, K * c.BROWS), min(hi, (K + 1) * c.BROWS)
            nc.sync.dma_start(
                shard_b[l][K][rlo - K * c.BROWS: rhi - K * c.BROWS, :],
                src_ap[rlo - lo: rhi - lo, :])
        while len(done_k) < c.NB and hi >= (len(done_k) + 1) * c.BROWS:
            emit_collective(l, len(done_k))
            done_k.append(len(done_k))

    with tile.TileContext(nc) as tc:
        with contextlib.ExitStack() as es:
            const = es.enter_context(tc.tile_pool(name="const", bufs=1))
            meta_p = es.enter_context(tc.tile_pool(name="meta", bufs=1))

            w1_sb = const.tile([D, D], BF16); nc.sync.dma_start(w1_sb[:], W1_in[:])
            w2_sb = const.tile([D, D], BF16); nc.sync.dma_start(w2_sb[:], W2_in[:])
            dinv_sb = const.tile([128, c.NW], F32)
            nc.sync.dma_start(dinv_sb[:], dinv_in[:])
            iota_sb = const.tile([128, 128], BF16)
            nc.sync.dma_start(iota_sb[:], iota_dram[:])
            ident_sb = const.tile([128, 128], BF16)
            nc.sync.dma_start(ident_sb[:], ident_dram[:])
            if use_b[1] or use_b[2]:
                b1_sb = const.tile([1, D], BF16)
                nc.sync.dma_start(b1_sb[:], b1_in[:])
                b2_sb = const.tile([1, D], BF16)
                nc.sync.dma_start(b2_sb[:], b2_in[:])
                invd_sb = const.tile([1, c.SHARD], BF16)
                nc.sync.dma_start(invd_sb[:], invd_in[:])
            gidx_sb = meta_p.tile([128, TOT // 16], I16)
            nc.sync.dma_start(gidx_sb[:], gidx_in[:])
            dstloc_sb = meta_p.tile([128, TOT_TILES], F32)
            nc.sync.dma_start(dstloc_sb[:], dstloc_in[:])
            negdst_sb = meta_p.tile([128, TOT_TILES], F32)
            nc.sync.dma_start(negdst_sb[:], negdst_in[:])

            span_base = {}
            tcol0 = 0
            for b in range(c.NBATCH):
                m0 = 0
                for g in range(c.NGB):
                    span_base[(b, g)] = (tcol0, m0)
                    tcol0 += int(span_tiles[b, g])
                    m0 += int(span_tiles[b, g])

            def emit_S(Sp, Ap, tcol):
                S = Sp.tile([128, 128], BF16, tag="S")
                if (tcol % 10) < c.ACT_TENTHS:
                    a = Ap.tile([128, 128], BF16, tag="a")
                    nc.scalar.activation(
                        a[:], iota_sb[:], ACTF.Abs,
                        bias=negdst_sb[:, tcol:tcol + 1])
                    nc.scalar.activation(
                        S[:], a[:], ACTF.Relu, scale=-1.0, bias=1.0)
                else:
                    nc.vector.tensor_scalar(
                        S[:], iota_sb[:], dstloc_sb[:, tcol:tcol + 1], None,
                        mybir.AluOpType.is_equal)
                return S

            pre = {}        # (lid, b) -> (Mt, set of emitted gbands)
            cc_waited = {1: set(), 2: set(), 3: set()}

            def batch_gathers(Mp, lid, b, only_avail=False):
                """Emit (remaining) gathers for batch b of layer lid."""
                Mt, done = pre.get((lid, b), (None, set()))
                if Mt is None:
                    Mt = Mp.tile([128, TBMAX, 128], BF16, tag="M")
                for g in range(c.NGB):
                    if g in done or int(span_tiles[b, g]) == 0:
                        continue
                    if only_avail and any((lid, K) not in ccst
                                          for K in GB_CHUNKS[g]):
                        continue
                    ntiles = int(span_tiles[b, g])
                    tb, mb = span_base[(b, g)]
                    deps = []
                    if g not in cc_waited[lid]:
                        deps = [ccst[(lid, K)] for K in GB_CHUNKS[g]]
                        cc_waited[lid].add(g)
                    emit_gather(
                        Mt[:, mb:mb + ntiles, :], tslice(lid, g),
                        gidx_sb[:, tb * 8:(tb + ntiles) * 8],
                        ntiles * 128, deps=deps)
                    done.add(g)
                pre[(lid, b)] = (Mt, done)
                return Mt

            def layer(Mp, lid, hall, hall_next, make_next, next_lid):
                waited_done = []
                with tc.tile_pool(name=f"S{lid}", bufs=32) as Sp, \
                     tc.tile_pool(name=f"A{lid}", bufs=8) as Ap, \
                     tc.tile_pool(name=f"ag{lid}", bufs=4, space="PSUM") as agp, \
                     tc.tile_pool(name=f"tp{lid}", bufs=2, space="PSUM") as tpp, \
                     tc.tile_pool(name=f"ep{lid}", bufs=4) as epp:
                    for b in range(c.NBATCH):
                        wlo, whi = b * c.WB, min((b + 1) * c.WB, c.NW)
                        Mt = batch_gathers(Mp, lid, b)
                        for w in range(wlo, whi):
                            ps = agp.tile([128, D], F32, tag="agg")
                            nmm = int(T[w].sum())
                            hsl = hall[:, w * 128:w * 128 + D]
                            nc.tensor.matmul(ps[:], lhsT=ident_sb[:],
                                             rhs=hsl,
                                             start=True, stop=(nmm == 0 and
                                                               not use_b[lid]))
                            mi = 0
                            for g in range(c.NGB):
                                tb, mb = span_base[(b, g)]
                                off = int(T[wlo:w, g].sum())
                                for t in range(int(T[w, g])):
                                    tcol = tb + off + t
                                    mcol = mb + off + t
                                    S = emit_S(Sp, Ap, tcol)
                                    mi += 1
                                    nc.tensor.matmul(
                                        ps[:], lhsT=S[:], rhs=Mt[:, mcol, :],
                                        start=False,
                                        stop=(mi == nmm and not use_b[lid]))
                            if use_b[lid]:
                                bsb = b1_sb if lid == 1 else b2_sb
                                nc.tensor.matmul(
                                    ps[:],
                                    lhsT=invd_sb[0:1, w * 128:(w + 1) * 128],
                                    rhs=bsb[:],
                                    start=False, stop=True)
                            if make_next:
                                z = epp.tile([128, D], BF16, tag="z")
                                nc.scalar.activation(
                                    z[:], ps[:], ACTF.Relu,
                                    scale=dinv_sb[:, w:w + 1])
                                zt_ps = tpp.tile([128, D], BF16, tag="zt")
                                nc.tensor.transpose(zt_ps[:], z[:], ident_sb[:])
                                zT = epp.tile([128, D], BF16, tag="zT")
                                nc.scalar.activation(zT[:], zt_ps[:], ACTF.Copy)
                                h2ps = tpp.tile([128, D], F32, tag="h2")
                                nc.tensor.matmul(h2ps[:], lhsT=zT[:],
                                                 rhs=w2_sb[:],
                                                 start=True, stop=True)
                                nc.scalar.activation(
                                    hall_next[:, w * 128:w * 128 + D], h2ps[:],
                                    ACTF.Copy, scale=dinv_sb[:, w:w + 1])
                            else:
                                nc.scalar.activation(
                                    hall_next[:, w * 128:w * 128 + D], ps[:],
                                    ACTF.Relu, scale=dinv_sb[:, w:w + 1])
                            write_window(next_lid, w,
                                         hall_next[:, w * 128:w * 128 + D],
                                         waited_done)
                        if b == c.NBATCH - 1 - c.PREFETCH and next_lid == 2:
                            for pb in range(min(c.PREFETCH + 1, c.NBATCH)):
                                batch_gathers(Mp, 2, pb, only_avail=True)

            with tc.tile_pool(name="hs", bufs=1) as hsp, \
                 tc.tile_pool(name="Mpool", bufs=3 + c.PREFETCH) as Mp:
                hs1_all = hsp.tile([128, c.SHARD], BF16, tag="hs1")
                hs2_all = hsp.tile([128, c.SHARD], BF16, tag="hs2")
                z_all = hsp.tile([128, c.SHARD], BF16, tag="z")
                # P0: hs1 = dinv * (x @ W1), xT resident
                done0 = []
                with tc.tile_pool(name="p0x", bufs=1) as p0x, \
                     tc.tile_pool(name="p0ps", bufs=4, space="PSUM") as p0ps:
                    xT_sb = p0x.tile([D, c.SHARD], BF16)
                    nc.sync.dma_start(xT_sb[:], xT_in[:])
                    for w in range(c.NW):
                        ps = p0ps.tile([128, D], F32, tag="ps")
                        nc.tensor.matmul(
                            ps[:], lhsT=xT_sb[:, w * 128:(w + 1) * 128],
                            rhs=w1_sb[:], start=True, stop=True)
                        nc.scalar.activation(
                            hs1_all[:, w * 128:w * 128 + D], ps[:],
                            ACTF.Copy, scale=dinv_sb[:, w:w + 1])
                        write_window(1, w, hs1_all[:, w * 128:w * 128 + D],
                                     done0)
                layer(Mp, 1, hs1_all, hs2_all, True, 2)
                layer(Mp, 2, hs2_all, z_all, False, 3)

            # decode
            with tc.tile_pool(name="didx", bufs=1) as didxp, \
                 tc.tile_pool(name="dM", bufs=1) as dMp, \
                 tc.tile_pool(name="dw", bufs=6) as dwp, \
                 tc.tile_pool(name="dout", bufs=1) as doutp:
                ds_sb = didxp.tile([128, TOT_DEC // 16], I16)
                nc.sync.dma_start(ds_sb[:], didx_s_in[:])
                dd_sb = didxp.tile([128, TOT_DEC // 16], I16)
                nc.sync.dma_start(dd_sb[:], didx_d_in[:])
                Ms = dMp.tile([128, TOT_DEC // 128, D], BF16, tag="Ms")
                Md = dMp.tile([128, TOT_DEC // 128, D], BF16, tag="Md")
                res = doutp.tile([128, TOT_DEC // 128], F32)
                waited = set()
                coff = 0
                for g in gorder:
                    ks, kd = g // c.NGB, g % c.NGB
                    ncols = int(Tdec[g])
                    if ncols == 0:
                        continue
                    dep_s, dep_d = [], []
                    if ks not in waited:
                        dep_s = [ccst[(3, K)] for K in GB_CHUNKS[ks]]
                        waited.add(ks)
                    if kd not in waited:
                        dep_d = [ccst[(3, K)] for K in GB_CHUNKS[kd]]
                        waited.add(kd)
                    off16 = coff * 8
                    emit_gather(Ms[:, coff:coff + ncols, :], tslice(3, ks),
                                ds_sb[:, off16:off16 + ncols * 8], ncols * 128,
                                deps=dep_s)
                    emit_gather(Md[:, coff:coff + ncols, :], tslice(3, kd),
                                dd_sb[:, off16:off16 + ncols * 8], ncols * 128,
                                deps=dep_d)
                    for t in range(ncols):
                        col = coff + t
                        mm = dwp.tile([128, D], F32, tag="mm")
                        nc.vector.tensor_tensor(
                            mm[:], Ms[:, col, :], Md[:, col, :],
                            op=mybir.AluOpType.mult)
                        trash = dwp.tile([128, D], BF16, tag="tr")
                        nc.scalar.activation(
                            trash[:], mm[:], ACTF.Copy,
                            accum_out=res[:, col:col + 1])
                    coff += ncols
                nc.sync.dma_start(dots_out[:], res[:])

    nc.compile()
    return nc


def assemble_output(cfg, meta, results):
    c = cfg
    slot2j = meta["slot2j"]
    out = np.zeros(c.EL, dtype=np.float32)
    for core in range(len(results)):
        d = np.asarray(results[core]["dots"], dtype=np.float32)
        flat = d.T.reshape(-1)             # slot i -> d[i%128, i//128]
        s2j = slot2j[core]
        valid = s2j >= 0
        out[s2j[valid]] = flat[valid]
    return out


def run_pipeline(x, edge_index, edge_label_index, W1, b1, W2, b2,
                 cfg=None, trace=False, tmpdir=None):
    cfg = cfg or DEFAULT
    in_maps, meta = host_prep(cfg, x, edge_index, edge_label_index,
                              W1, b1, W2, b2)
    nc = build_program(cfg, meta)
    res = run_bass_kernel_spmd(nc, in_maps, list(range(cfg.NC)),
                               trace=trace, tmpdir=tmpdir)
    return assemble_output(cfg, meta, res.results), res


def kernel(x, edge_index, edge_label_index, W1, b1, W2, b2):
    out, _ = run_pipeline(x, edge_index, edge_label_index, W1, b1, W2, b2)
    return out


# revision 16
# speedup vs baseline: 1.2013x; 1.2013x over previous
"""Trainium2 Bass kernel for nn_LinkPredictor (2-layer GCN + edge-dot decode).

Strategy (8 NeuronCores, SPMD), v3:
  - Nodes sharded: core c owns rows [c*12544, (c+1)*12544) of the padded
    node table (N=100000 padded to 100352 = 8*98*128).
  - dinv folded into node features: table rows hold hs = dinv[n] * (prev @ W);
    output z = relu(dinv[v]*(agg + hs[v]) + b).
  - Node tables in DRAM are band-major contiguous [4*25088, D]: collective
    chunk K holds rows c*3136+j of each core's shard; 4 pipelined AllGathers
    per layer write slices of one tensor.
  - Gathers use signed-int16 indices with offset bases (hardware sign-extends)
    so only 2 gather-bands cover all 100352 rows -> bigger edge groups,
    ~10% less padding. One dma_gather per (window-batch, gather-band).
  - Aggregation: pure one-hot S (single-op DVE tensor_scalar is_equal, dead
    slots dstloc=-1; a fraction of tiles built on ScalarE via Abs+Relu)
    feeding PE matmul accumulation into PSUM. Self-loop = identity matmul;
    bias = rank-1 matmul (skipped when b==0); relu+dinv scale on ScalarE.
  - xT and hs tiles resident in SBUF as wide [128, 12544] tiles.
  - Next layer's first batch (gather-band A) is prefetched during the
    current layer's tail through a shared M pool.
  - Decode: gathers z[s], z[d] by gather-band pair, DVE mult + ACT accum.
"""
import contextlib
import math
import numpy as np
import ml_dtypes

import concourse.bass as bass
import concourse.tile as tile
from concourse import bacc, mybir
from concourse.bass_utils import run_bass_kernel_spmd
from concourse.tile_rust import add_dep_helper

F32 = mybir.dt.float32
BF16 = mybir.dt.bfloat16
I16 = mybir.dt.int16
BF = ml_dtypes.bfloat16
ACTF = mybir.ActivationFunctionType


class Cfg:
    def __init__(self, N=100000, E=1600000, EL=100000, D=128, ncores=8,
                 nw=98, nbands=4, wb=3, act_tenths=3, prefetch=1):
        self.N, self.E, self.EL, self.D, self.NC = N, E, EL, D, ncores
        self.NW = nw                      # windows (128 nodes each) per core
        self.SHARD = nw * 128             # nodes per core (padded)
        self.NP = self.SHARD * ncores     # padded node count
        assert self.NP >= N
        self.NB = nbands                  # collective chunks per shard
        assert self.SHARD % nbands == 0
        self.BROWS = self.SHARD // nbands # shard rows per collective chunk
        self.TBROWS = self.BROWS * ncores # table rows per chunk (band-major)
        self.NGB = 4                      # gather bands (positive int16)
        self.GBASE = [0, 25088, 50176, 75264]  # == collective chunk starts
        self.WB = wb                      # windows per gather/aggregate batch
        self.NBATCH = math.ceil(nw / wb)
        self.ACT_TENTHS = act_tenths      # S-build tiles on ScalarE /10
        self.PREFETCH = prefetch          # next-layer batches gathered early


DEFAULT = Cfg()


def _wrap_idxs(idx):
    """[n] ints -> [128, n//16] int16 wrapped in 16 partitions, replicated 8x."""
    n = len(idx)
    assert n % 16 == 0
    w = np.asarray(idx, dtype=np.int16).reshape(n // 16, 16).T
    return np.ascontiguousarray(np.tile(w, (8, 1)))


def host_prep(cfg, x, edge_index, edge_label_index, W1, b1, W2, b2):
    """All host-side sharding/layout. Returns (per-core input maps, meta)."""
    c = cfg
    src = np.asarray(edge_index[0], dtype=np.int64)
    dst = np.asarray(edge_index[1], dtype=np.int64)
    deg = np.bincount(dst, minlength=c.N).astype(np.float64) + 1.0
    dinv = 1.0 / np.sqrt(deg)                      # [N]
    dinv_p = np.ones(c.NP, dtype=np.float64)
    dinv_p[:c.N] = dinv
    dinv_f = dinv_p.astype(np.float32)

    def bmaj_of(n):
        """band-major table row of node id n."""
        cc, r = n // c.SHARD, n % c.SHARD
        K = r // c.BROWS
        return K * c.TBROWS + cc * c.BROWS + (r - K * c.BROWS)

    bmaj_src = bmaj_of(src)
    gband_src = bmaj_src // c.TBROWS
    core_of = dst // c.SHARD
    w_of = (dst % c.SHARD) // 128
    dloc = dst % 128

    key = (core_of * c.NW + w_of) * c.NGB + gband_src
    ngroups = c.NC * c.NW * c.NGB
    order = np.argsort(key, kind="stable")
    counts = np.bincount(key, minlength=ngroups).reshape(c.NC, c.NW, c.NGB)
    starts = np.zeros(ngroups + 1, dtype=np.int64)
    np.cumsum(np.bincount(key, minlength=ngroups), out=starts[1:])

    T = np.ceil(counts.max(axis=0) / 128).astype(np.int64)     # [NW, NGB]
    TOT_TILES = int(T.sum())
    span_tiles = np.zeros((c.NBATCH, c.NGB), dtype=np.int64)
    for b in range(c.NBATCH):
        wlo, whi = b * c.WB, min((b + 1) * c.WB, c.NW)
        for g in range(c.NGB):
            span_tiles[b, g] = T[wlo:whi, g].sum()
    TOT = TOT_TILES * 128

    idx_arr = np.zeros((c.NC, TOT), dtype=np.int64)
    dloc_arr = np.full((c.NC, TOT), -1.0, dtype=np.float32)
    for core in range(c.NC):
        pos = 0
        for b in range(c.NBATCH):
            wlo, whi = b * c.WB, min((b + 1) * c.WB, c.NW)
            for g in range(c.NGB):
                for w in range(wlo, whi):
                    gk = (core * c.NW + w) * c.NGB + g
                    eids = order[starts[gk]:starts[gk + 1]]
                    n = len(eids)
                    idx_arr[core, pos:pos + n] = bmaj_src[eids] - c.GBASE[g]
                    dloc_arr[core, pos:pos + n] = dloc[eids]
                    pos += int(T[w, g]) * 128
        assert pos == TOT
    assert idx_arr.min() >= 0 and idx_arr.max() < c.TBROWS

    # decode: label edge j -> core j // ELC; groups by (gband(s), gband(d))
    assert c.EL % c.NC == 0
    ELC = c.EL // c.NC
    ls = np.asarray(edge_label_index[0], dtype=np.int64)
    ld = np.asarray(edge_label_index[1], dtype=np.int64)
    bs, bd = bmaj_of(ls), bmaj_of(ld)
    gs = bs // c.TBROWS
    gd = bd // c.TBROWS
    gdec = gs * c.NGB + gd
    NG_DEC = c.NGB * c.NGB
    cnt_dec = np.zeros((c.NC, NG_DEC), dtype=np.int64)
    for core in range(c.NC):
        cnt_dec[core] = np.bincount(gdec[core * ELC:(core + 1) * ELC],
                                    minlength=NG_DEC)
    Tdec = np.ceil(cnt_dec.max(axis=0) / 128).astype(np.int64)   # [NG_DEC]
    gorder = sorted(range(NG_DEC), key=lambda g: (max(g // c.NGB, g % c.NGB), g))
    TOT_DEC = int(Tdec.sum()) * 128
    idx_s = np.zeros((c.NC, TOT_DEC), dtype=np.int64)
    idx_d = np.zeros((c.NC, TOT_DEC), dtype=np.int64)
    slot2j = np.full((c.NC, TOT_DEC), -1, dtype=np.int64)
    for core in range(c.NC):
        jlo = core * ELC
        kk = gdec[jlo:jlo + ELC]
        o = np.argsort(kk, kind="stable")
        st = np.zeros(NG_DEC + 1, dtype=np.int64)
        np.cumsum(np.bincount(kk, minlength=NG_DEC), out=st[1:])
        pos = 0
        for g in gorder:
            js = o[st[g]:st[g + 1]] + jlo
            n = len(js)
            idx_s[core, pos:pos + n] = bs[js] - c.GBASE[g // c.NGB]
            idx_d[core, pos:pos + n] = bd[js] - c.GBASE[g % c.NGB]
            slot2j[core, pos:pos + n] = js
            pos += int(Tdec[g]) * 128
        assert pos == TOT_DEC

    xp = np.zeros((c.NP, c.D), dtype=np.float32)
    xp[:c.N] = np.asarray(x, dtype=np.float32)
    use_b1 = bool(np.any(np.asarray(b1)))
    use_b2 = bool(np.any(np.asarray(b2)))

    in_maps = []
    for core in range(c.NC):
        sl = slice(core * c.SHARD, (core + 1) * c.SHARD)
        dsh = dinv_f[sl]
        m = {
            "xT": np.ascontiguousarray(xp[sl].T).astype(BF),
            "W1": np.asarray(W1, dtype=np.float32).astype(BF),
            "W2": np.asarray(W2, dtype=np.float32).astype(BF),
            "dinv": np.ascontiguousarray(dsh.reshape(c.NW, 128).T),
            "gidx": _wrap_idxs(idx_arr[core]),
            "dstloc": np.ascontiguousarray(
                dloc_arr[core].reshape(TOT_TILES, 128).T),
            "negdst": np.ascontiguousarray(
                (-dloc_arr[core]).reshape(TOT_TILES, 128).T),
            "didx_s": _wrap_idxs(idx_s[core]),
            "didx_d": _wrap_idxs(idx_d[core]),
        }
        if use_b1 or use_b2:
            m["b1r"] = np.asarray(b1, np.float32)[None, :].astype(BF)
            m["b2r"] = np.asarray(b2, np.float32)[None, :].astype(BF)
            m["invd"] = (1.0 / dsh)[None, :].astype(BF)
        in_maps.append(m)
    meta = dict(T=T, span_tiles=span_tiles, TOT=TOT, TOT_TILES=TOT_TILES,
                Tdec=Tdec, gorder=gorder, TOT_DEC=TOT_DEC, slot2j=slot2j,
                use_b1=use_b1, use_b2=use_b2)
    return in_maps, meta


def build_program(cfg, meta, num_cores=None):
    c = cfg
    NCores = num_cores or c.NC
    T, span_tiles = meta["T"], meta["span_tiles"]
    TOT, TOT_TILES = meta["TOT"], meta["TOT_TILES"]
    Tdec, gorder, TOT_DEC = meta["Tdec"], meta["gorder"], meta["TOT_DEC"]
    use_b = {1: meta["use_b1"], 2: meta["use_b2"]}
    D = c.D
    TBMAX = int(span_tiles.sum(axis=1).max())
    # gather band g reads exactly collective chunk g (1:1)
    GB_CHUNKS = [[g] for g in range(c.NGB)]

    nc = bacc.Bacc("TRN2", target_bir_lowering=False, debug=False,
                   num_devices=NCores, num_swdge_queues=4)
    NQ = 4

    xT_in = nc.dram_tensor("xT", [D, c.SHARD], BF16, kind="ExternalInput")
    W1_in = nc.dram_tensor("W1", [D, D], BF16, kind="ExternalInput")
    W2_in = nc.dram_tensor("W2", [D, D], BF16, kind="ExternalInput")
    dinv_in = nc.dram_tensor("dinv", [128, c.NW], F32, kind="ExternalInput")
    gidx_in = nc.dram_tensor("gidx", [128, TOT // 16], I16, kind="ExternalInput")
    dstloc_in = nc.dram_tensor("dstloc", [128, TOT_TILES], F32, kind="ExternalInput")
    negdst_in = nc.dram_tensor("negdst", [128, TOT_TILES], F32, kind="ExternalInput")
    didx_s_in = nc.dram_tensor("didx_s", [128, TOT_DEC // 16], I16, kind="ExternalInput")
    didx_d_in = nc.dram_tensor("didx_d", [128, TOT_DEC // 16], I16, kind="ExternalInput")
    if use_b[1] or use_b[2]:
        b1_in = nc.dram_tensor("b1r", [1, D], BF16, kind="ExternalInput")
        b2_in = nc.dram_tensor("b2r", [1, D], BF16, kind="ExternalInput")
        invd_in = nc.dram_tensor("invd", [1, c.SHARD], BF16, kind="ExternalInput")
    dots_out = nc.dram_tensor("dots", [128, TOT_DEC // 128], F32, kind="ExternalOutput")

    shard_b = {l: [nc.dram_tensor(f"shard{l}_{k}", [c.BROWS, D], BF16)
                   for k in range(c.NB)] for l in (1, 2, 3)}
    table = {l: nc.dram_tensor(f"table{l}", [c.NB * c.TBROWS, D], BF16,
                               addr_space="Shared") for l in (1, 2, 3)}

    def tslice(l, g):
        base = c.GBASE[g]
        return table[l][base:base + c.TBROWS, :]

    iota_dram = nc.inline_tensor(
        np.tile(np.arange(128, dtype=np.float32), (128, 1)).astype(BF), "iota_c")
    ident_dram = nc.inline_tensor(np.eye(128, dtype=np.float32).astype(BF), "ident_c")

    core_ids = list(range(NCores))
    gst = {"count": 0, "prev": None}
    ccst = {}                            # (l, K) -> collective instruction

    def emit_gather(out_ap, in_ap, idx_ap, n_idx, deps=()):
        q = gst["count"] % NQ
        inst = nc.gpsimd.dma_gather(out_ap, in_ap, idx_ap, n_idx, n_idx, D,
                                    queue_num=q, single_packet=False)
        if gst["prev"] is not None:
            add_dep_helper(inst.ins, gst["prev"].ins, sync=False,
                           reason="pin swdge queue order")
        for dcc in deps:
            add_dep_helper(inst.ins, dcc.ins, sync=True,
                           reason="gather after collective")
        gst["prev"] = inst
        gst["count"] += 1
        return inst

    def emit_collective(l, K):
        cc = nc.gpsimd.collective_compute(
            "AllGather", mybir.AluOpType.bypass,
            replica_groups=[core_ids],
            ins=[shard_b[l][K][:]],
            outs=[table[l][K * c.TBROWS:(K + 1) * c.TBROWS, :]],
        )
        if gst["prev"] is not None:
            add_dep_helper(cc.ins, gst["prev"].ins, sync=False,
                           reason="order on gpsimd")
        gst["prev"] = cc
        ccst[(l, K)] = cc
        return cc

    def write_window(l, w, src_ap, done_k):
        """DMA window w rows into banded shards; fire collectives when a
        band completes."""
        lo, hi = w * 128, (w + 1) * 128
        k0, k1 = lo // c.BROWS, (hi - 1) // c.BROWS
        for K in range(k0, k1 + 1):
            rlo, rhi = max(lo, K * c.BROWS), min(hi, (K + 1) * c.BROWS)
            nc.sync.dma_start(
                shard_b[l][K][rlo - K * c.BROWS: rhi - K * c.BROWS, :],
                src_ap[rlo - lo: rhi - lo, :])
        while len(done_k) < c.NB and hi >= (len(done_k) + 1) * c.BROWS:
            emit_collective(l, len(done_k))
            done_k.append(len(done_k))

    with tile.TileContext(nc) as tc:
        with contextlib.ExitStack() as es:
            const = es.enter_context(tc.tile_pool(name="const", bufs=1))
            meta_p = es.enter_context(tc.tile_pool(name="meta", bufs=1))

            w1_sb = const.tile([D, D], BF16); nc.sync.dma_start(w1_sb[:], W1_in[:])
            w2_sb = const.tile([D, D], BF16); nc.sync.dma_start(w2_sb[:], W2_in[:])
            dinv_sb = const.tile([128, c.NW], F32)
            nc.sync.dma_start(dinv_sb[:], dinv_in[:])
            iota_sb = const.tile([128, 128], BF16)
            nc.sync.dma_start(iota_sb[:], iota_dram[:])
            ident_sb = const.tile([128, 128], BF16)
            nc.sync.dma_start(ident_sb[:], ident_dram[:])
            if use_b[1] or use_b[2]:
                b1_sb = const.tile([1, D], BF16)
                nc.sync.dma_start(b1_sb[:], b1_in[:])
                b2_sb = const.tile([1, D], BF16)
                nc.sync.dma_start(b2_sb[:], b2_in[:])
                invd_sb = const.tile([1, c.SHARD], BF16)
                nc.sync.dma_start(invd_sb[:], invd_in[:])
            gidx_sb = meta_p.tile([128, TOT // 16], I16)
            nc.sync.dma_start(gidx_sb[:], gidx_in[:])
            dstloc_sb = meta_p.tile([128, TOT_TILES], F32)
            nc.sync.dma_start(dstloc_sb[:], dstloc_in[:])
            negdst_sb = meta_p.tile([128, TOT_TILES], F32)
            nc.sync.dma_start(negdst_sb[:], negdst_in[:])

            span_base = {}
            tcol0 = 0
            for b in range(c.NBATCH):
                m0 = 0
                for g in range(c.NGB):
                    span_base[(b, g)] = (tcol0, m0)
                    tcol0 += int(span_tiles[b, g])
                    m0 += int(span_tiles[b, g])

            def emit_S(Sp, Ap, tcol):
                S = Sp.tile([128, 128], BF16, tag="S")
                if (tcol % 10) < c.ACT_TENTHS:
                    a = Ap.tile([128, 128], BF16, tag="a")
                    nc.scalar.activation(
                        a[:], iota_sb[:], ACTF.Abs,
                        bias=negdst_sb[:, tcol:tcol + 1])
                    nc.scalar.activation(
                        S[:], a[:], ACTF.Relu, scale=-1.0, bias=1.0)
                else:
                    nc.vector.tensor_scalar(
                        S[:], iota_sb[:], dstloc_sb[:, tcol:tcol + 1], None,
                        mybir.AluOpType.is_equal)
                return S

            pre = {}        # (lid, b) -> (Mt, set of emitted gbands)
            cc_waited = {1: set(), 2: set(), 3: set()}

            def batch_gathers(Mp, lid, b, only_avail=False):
                """Emit (remaining) gathers for batch b of layer lid."""
                Mt, done = pre.get((lid, b), (None, set()))
                if Mt is None:
                    Mt = Mp.tile([128, TBMAX, 128], BF16, tag="M")
                for g in range(c.NGB):
                    if g in done or int(span_tiles[b, g]) == 0:
                        continue
                    if only_avail and any((lid, K) not in ccst
                                          for K in GB_CHUNKS[g]):
                        continue
                    ntiles = int(span_tiles[b, g])
                    tb, mb = span_base[(b, g)]
                    deps = []
                    if g not in cc_waited[lid]:
                        deps = [ccst[(lid, K)] for K in GB_CHUNKS[g]]
                        cc_waited[lid].add(g)
                    emit_gather(
                        Mt[:, mb:mb + ntiles, :], tslice(lid, g),
                        gidx_sb[:, tb * 8:(tb + ntiles) * 8],
                        ntiles * 128, deps=deps)
                    done.add(g)
                pre[(lid, b)] = (Mt, done)
                return Mt

            def layer(Mp, lid, hall, hall_next, make_next, next_lid):
                waited_done = []
                with tc.tile_pool(name=f"S{lid}", bufs=32) as Sp, \
                     tc.tile_pool(name=f"A{lid}", bufs=8) as Ap, \
                     tc.tile_pool(name=f"ag{lid}", bufs=4, space="PSUM") as agp, \
                     tc.tile_pool(name=f"tp{lid}", bufs=2, space="PSUM") as tpp, \
                     tc.tile_pool(name=f"ep{lid}", bufs=4) as epp:
                    for b in range(c.NBATCH):
                        wlo, whi = b * c.WB, min((b + 1) * c.WB, c.NW)
                        Mt = batch_gathers(Mp, lid, b)
                        for w in range(wlo, whi):
                            ps = agp.tile([128, D], F32, tag="agg")
                            nmm = int(T[w].sum())
                            hsl = hall[:, w * 128:w * 128 + D]
                            nc.tensor.matmul(ps[:], lhsT=ident_sb[:],
                                             rhs=hsl,
                                             start=True, stop=(nmm == 0 and
                                                               not use_b[lid]))
                            mi = 0
                            for g in range(c.NGB):
                                tb, mb = span_base[(b, g)]
                                off = int(T[wlo:w, g].sum())
                                for t in range(int(T[w, g])):
                                    tcol = tb + off + t
                                    mcol = mb + off + t
                                    S = emit_S(Sp, Ap, tcol)
                                    mi += 1
                                    nc.tensor.matmul(
                                        ps[:], lhsT=S[:], rhs=Mt[:, mcol, :],
                                        start=False,
                                        stop=(mi == nmm and not use_b[lid]))
                            if use_b[lid]:
                                bsb = b1_sb if lid == 1 else b2_sb
                                nc.tensor.matmul(
                                    ps[:],
                                    lhsT=invd_sb[0:1, w * 128:(w + 1) * 128],
                                    rhs=bsb[:],
                                    start=False, stop=True)
                            if make_next:
                                z = epp.tile([128, D], BF16, tag="z")
                                nc.scalar.activation(
                                    z[:], ps[:], ACTF.Relu,
                                    scale=dinv_sb[:, w:w + 1])
                                zt_ps = tpp.tile([128, D], BF16, tag="zt")
                                nc.tensor.transpose(zt_ps[:], z[:], ident_sb[:])
                                zT = epp.tile([128, D], BF16, tag="zT")
                                nc.scalar.activation(zT[:], zt_ps[:], ACTF.Copy)
                                h2ps = tpp.tile([128, D], F32, tag="h2")
                                nc.tensor.matmul(h2ps[:], lhsT=zT[:],
                                                 rhs=w2_sb[:],
                                                 start=True, stop=True)
                                nc.scalar.activation(
                                    hall_next[:, w * 128:w * 128 + D], h2ps[:],
                                    ACTF.Copy, scale=dinv_sb[:, w:w + 1])
                            else:
                                nc.scalar.activation(
                                    hall_next[:, w * 128:w * 128 + D], ps[:],
                                    ACTF.Relu, scale=dinv_sb[:, w:w + 1])
                            write_window(next_lid, w,
                                         hall_next[:, w * 128:w * 128 + D],
                                         waited_done)
                        if b == c.NBATCH - 1 - c.PREFETCH and next_lid == 2:
                            for pb in range(min(c.PREFETCH + 1, c.NBATCH)):
                                batch_gathers(Mp, 2, pb, only_avail=True)

            with tc.tile_pool(name="hs", bufs=1) as hsp, \
                 tc.tile_pool(name="Mpool", bufs=3 + c.PREFETCH) as Mp:
                hs1_all = hsp.tile([128, c.SHARD], BF16, tag="hs1")
                hs2_all = hsp.tile([128, c.SHARD], BF16, tag="hs2")
                z_all = hsp.tile([128, c.SHARD], BF16, tag="z")
                # P0: hs1 = dinv * (x @ W1), xT resident
                done0 = []
                with tc.tile_pool(name="p0x", bufs=1) as p0x, \
                     tc.tile_pool(name="p0ps", bufs=4, space="PSUM") as p0ps:
                    xT_sb = p0x.tile([D, c.SHARD], BF16)
                    nc.sync.dma_start(xT_sb[:], xT_in[:])
                    for w in range(c.NW):
                        ps = p0ps.tile([128, D], F32, tag="ps")
                        nc.tensor.matmul(
                            ps[:], lhsT=xT_sb[:, w * 128:(w + 1) * 128],
                            rhs=w1_sb[:], start=True, stop=True)
                        nc.scalar.activation(
                            hs1_all[:, w * 128:w * 128 + D], ps[:],
                            ACTF.Copy, scale=dinv_sb[:, w:w + 1])
                        write_window(1, w, hs1_all[:, w * 128:w * 128 + D],
                                     done0)
                layer(Mp, 1, hs1_all, hs2_all, True, 2)
                layer(Mp, 2, hs2_all, z_all, False, 3)

            # decode
            with tc.tile_pool(name="didx", bufs=1) as didxp, \
                 tc.tile_pool(name="dM", bufs=1) as dMp, \
                 tc.tile_pool(name="dw", bufs=6) as dwp, \
                 tc.tile_pool(name="dout", bufs=1) as doutp:
                ds_sb = didxp.tile([128, TOT_DEC // 16], I16)
                nc.sync.dma_start(ds_sb[:], didx_s_in[:])
                dd_sb = didxp.tile([128, TOT_DEC // 16], I16)
                nc.sync.dma_start(dd_sb[:], didx_d_in[:])
                Ms = dMp.tile([128, TOT_DEC // 128, D], BF16, tag="Ms")
                Md = dMp.tile([128, TOT_DEC // 128, D], BF16, tag="Md")
                res = doutp.tile([128, TOT_DEC // 128], F32)
                waited = set()
                coff = 0
                for g in gorder:
                    ks, kd = g // c.NGB, g % c.NGB
                    ncols = int(Tdec[g])
                    if ncols == 0:
                        continue
                    dep_s, dep_d = [], []
                    if ks not in waited:
                        dep_s = [ccst[(3, K)] for K in GB_CHUNKS[ks]]
                        waited.add(ks)
                    if kd not in waited:
                        dep_d = [ccst[(3, K)] for K in GB_CHUNKS[kd]]
                        waited.add(kd)
                    off16 = coff * 8
                    emit_gather(Ms[:, coff:coff + ncols, :], tslice(3, ks),
                                ds_sb[:, off16:off16 + ncols * 8], ncols * 128,
                                deps=dep_s)
                    emit_gather(Md[:, coff:coff + ncols, :], tslice(3, kd),
                                dd_sb[:, off16:off16 + ncols * 8], ncols * 128,
                                deps=dep_d)
                    for t in range(ncols):
                        col = coff + t
                        mm = dwp.tile([128, D], F32, tag="mm")
                        nc.vector.tensor_tensor(
                            mm[:], Ms[:, col, :], Md[:, col, :],
                            op=mybir.AluOpType.mult)
                        trash = dwp.tile([128, D], BF16, tag="tr")
                        nc.scalar.activation(
                            trash[:], mm[:], ACTF.Copy,
                            accum_out=res[:, col:col + 1])
                    coff += ncols
                nc.sync.dma_start(dots_out[:], res[:])

    nc.compile()
    return nc


def assemble_output(cfg, meta, results):
    c = cfg
    slot2j = meta["slot2j"]
    out = np.zeros(c.EL, dtype=np.float32)
    for core in range(len(results)):
        d = np.asarray(results[core]["dots"], dtype=np.float32)
        flat = d.T.reshape(-1)             # slot i -> d[i%128, i//128]
        s2j = slot2j[core]
        valid = s2j >= 0
        out[s2j[valid]] = flat[valid]
    return out


def run_pipeline(x, edge_index, edge_label_index, W1, b1, W2, b2,
                 cfg=None, trace=False, tmpdir=None):
    cfg = cfg or DEFAULT
    in_maps, meta = host_prep(cfg, x, edge_index, edge_label_index,
                              W1, b1, W2, b2)
    nc = build_program(cfg, meta)
    res = run_bass_kernel_spmd(nc, in_maps, list(range(cfg.NC)),
                               trace=trace, tmpdir=tmpdir)
    return assemble_output(cfg, meta, res.results), res


def kernel(x, edge_index, edge_label_index, W1, b1, W2, b2):
    out, _ = run_pipeline(x, edge_index, edge_label_index, W1, b1, W2, b2)
    return out


# revision 18
# speedup vs baseline: 1.3625x; 1.1342x over previous
"""Trainium2 Bass kernel for nn_LinkPredictor (2-layer GCN + edge-dot decode).

Strategy (8 NeuronCores, SPMD), v3:
  - Nodes sharded: core c owns rows [c*12544, (c+1)*12544) of the padded
    node table (N=100000 padded to 100352 = 8*98*128).
  - dinv folded into node features: table rows hold hs = dinv[n] * (prev @ W);
    output z = relu(dinv[v]*(agg + hs[v]) + b).
  - Node tables in DRAM are band-major contiguous [4*25088, D]: collective
    chunk K holds rows c*3136+j of each core's shard; 4 pipelined AllGathers
    per layer write slices of one tensor.
  - Gathers use signed-int16 indices with offset bases (hardware sign-extends)
    so only 2 gather-bands cover all 100352 rows -> bigger edge groups,
    ~10% less padding. One dma_gather per (window-batch, gather-band).
  - Aggregation: pure one-hot S (single-op DVE tensor_scalar is_equal, dead
    slots dstloc=-1; a fraction of tiles built on ScalarE via Abs+Relu)
    feeding PE matmul accumulation into PSUM. Self-loop = identity matmul;
    bias = rank-1 matmul (skipped when b==0); relu+dinv scale on ScalarE.
  - xT and hs tiles resident in SBUF as wide [128, 12544] tiles.
  - Next layer's first batch (gather-band A) is prefetched during the
    current layer's tail through a shared M pool.
  - Decode: gathers z[s], z[d] by gather-band pair, DVE mult + ACT accum.
"""
import contextlib
import math
import numpy as np
import ml_dtypes

import concourse.bass as bass
import concourse.tile as tile
from concourse import bacc, mybir
from concourse.bass_utils import run_bass_kernel_spmd
from concourse.tile_rust import add_dep_helper

F32 = mybir.dt.float32
BF16 = mybir.dt.bfloat16
I16 = mybir.dt.int16
BF = ml_dtypes.bfloat16
ACTF = mybir.ActivationFunctionType


class Cfg:
    def __init__(self, N=100000, E=1600000, EL=100000, D=128, ncores=8,
                 nw=98, nbands=4, wb=3, act_tenths=2, prefetch=1):
        self.N, self.E, self.EL, self.D, self.NC = N, E, EL, D, ncores
        self.NW = nw                      # windows (128 nodes each) per core
        self.SHARD = nw * 128             # nodes per core (padded)
        self.NP = self.SHARD * ncores     # padded node count
        assert self.NP >= N
        # collective chunks (rows per core), window-aligned, small tail so
        # the last collective after the final window is short
        self.CH_WIN = [28, 28, 28, 14]    # windows per chunk
        assert sum(self.CH_WIN) == nw
        self.NB = len(self.CH_WIN)
        self.CH_SIZES = [wn * 128 for wn in self.CH_WIN]
        self.CH_STARTS = np.cumsum([0] + self.CH_SIZES).tolist()
        self.TB_SIZES = [s * ncores for s in self.CH_SIZES]
        self.TB_STARTS = np.cumsum([0] + self.TB_SIZES).tolist()
        assert max(self.TB_SIZES) <= 32768
        self.NGB = self.NB                # gather bands == chunks (1:1)
        self.GBASE = self.TB_STARTS[:-1]
        self.WB = wb                      # windows per gather/aggregate batch
        self.NBATCH = math.ceil(nw / wb)
        self.ACT_TENTHS = act_tenths      # S-build tiles on ScalarE /10
        self.PREFETCH = prefetch          # next-layer batches gathered early


DEFAULT = Cfg()


def _wrap_idxs(idx):
    """[n] ints -> [128, n//16] int16 wrapped in 16 partitions, replicated 8x."""
    n = len(idx)
    assert n % 16 == 0
    w = np.asarray(idx, dtype=np.int16).reshape(n // 16, 16).T
    return np.ascontiguousarray(np.tile(w, (8, 1)))


def host_prep(cfg, x, edge_index, edge_label_index, W1, b1, W2, b2):
    """All host-side sharding/layout. Returns (per-core input maps, meta)."""
    c = cfg
    src = np.asarray(edge_index[0], dtype=np.int64)
    dst = np.asarray(edge_index[1], dtype=np.int64)
    deg = np.bincount(dst, minlength=c.N).astype(np.float64) + 1.0
    dinv = 1.0 / np.sqrt(deg)                      # [N]
    dinv_p = np.ones(c.NP, dtype=np.float64)
    dinv_p[:c.N] = dinv
    dinv_f = dinv_p.astype(np.float32)

    ch_starts = np.asarray(c.CH_STARTS[:-1])
    def chunk_of(r):
        return np.searchsorted(ch_starts, r, side="right") - 1

    def bmaj_of(n):
        """band-major table row of node id n."""
        cc, r = n // c.SHARD, n % c.SHARD
        K = chunk_of(r)
        return (np.asarray(c.TB_STARTS)[K] + cc * np.asarray(c.CH_SIZES)[K]
                + (r - ch_starts[K]))

    bmaj_src = bmaj_of(src)
    gband_src = chunk_of(src % c.SHARD)
    core_of = dst // c.SHARD
    w_of = (dst % c.SHARD) // 128
    dloc = dst % 128

    key = (core_of * c.NW + w_of) * c.NGB + gband_src
    ngroups = c.NC * c.NW * c.NGB
    order = np.argsort(key, kind="stable")
    counts = np.bincount(key, minlength=ngroups).reshape(c.NC, c.NW, c.NGB)
    starts = np.zeros(ngroups + 1, dtype=np.int64)
    np.cumsum(np.bincount(key, minlength=ngroups), out=starts[1:])

    T = np.ceil(counts.max(axis=0) / 128).astype(np.int64)     # [NW, NGB]
    TOT_TILES = int(T.sum())
    span_tiles = np.zeros((c.NBATCH, c.NGB), dtype=np.int64)
    for b in range(c.NBATCH):
        wlo, whi = b * c.WB, min((b + 1) * c.WB, c.NW)
        for g in range(c.NGB):
            span_tiles[b, g] = T[wlo:whi, g].sum()
    TOT = TOT_TILES * 128

    idx_arr = np.zeros((c.NC, TOT), dtype=np.int64)
    dloc_arr = np.full((c.NC, TOT), -1.0, dtype=np.float32)
    for core in range(c.NC):
        pos = 0
        for b in range(c.NBATCH):
            wlo, whi = b * c.WB, min((b + 1) * c.WB, c.NW)
            for g in range(c.NGB):
                for w in range(wlo, whi):
                    gk = (core * c.NW + w) * c.NGB + g
                    eids = order[starts[gk]:starts[gk + 1]]
                    n = len(eids)
                    idx_arr[core, pos:pos + n] = bmaj_src[eids] - c.GBASE[g]
                    dloc_arr[core, pos:pos + n] = dloc[eids]
                    pos += int(T[w, g]) * 128
        assert pos == TOT
    assert idx_arr.min() >= 0 and idx_arr.max() < 32768

    # decode: label edge j -> core j // ELC; groups by (gband(s), gband(d))
    assert c.EL % c.NC == 0
    ELC = c.EL // c.NC
    ls = np.asarray(edge_label_index[0], dtype=np.int64)
    ld = np.asarray(edge_label_index[1], dtype=np.int64)
    bs, bd = bmaj_of(ls), bmaj_of(ld)
    gs = chunk_of(ls % c.SHARD)
    gd = chunk_of(ld % c.SHARD)
    gdec = gs * c.NGB + gd
    NG_DEC = c.NGB * c.NGB
    cnt_dec = np.zeros((c.NC, NG_DEC), dtype=np.int64)
    for core in range(c.NC):
        cnt_dec[core] = np.bincount(gdec[core * ELC:(core + 1) * ELC],
                                    minlength=NG_DEC)
    Tdec = np.ceil(cnt_dec.max(axis=0) / 128).astype(np.int64)   # [NG_DEC]
    gorder = sorted(range(NG_DEC), key=lambda g: (max(g // c.NGB, g % c.NGB), g))
    TOT_DEC = int(Tdec.sum()) * 128
    idx_s = np.zeros((c.NC, TOT_DEC), dtype=np.int64)
    idx_d = np.zeros((c.NC, TOT_DEC), dtype=np.int64)
    slot2j = np.full((c.NC, TOT_DEC), -1, dtype=np.int64)
    for core in range(c.NC):
        jlo = core * ELC
        kk = gdec[jlo:jlo + ELC]
        o = np.argsort(kk, kind="stable")
        st = np.zeros(NG_DEC + 1, dtype=np.int64)
        np.cumsum(np.bincount(kk, minlength=NG_DEC), out=st[1:])
        pos = 0
        for g in gorder:
            js = o[st[g]:st[g + 1]] + jlo
            n = len(js)
            idx_s[core, pos:pos + n] = bs[js] - c.GBASE[g // c.NGB]
            idx_d[core, pos:pos + n] = bd[js] - c.GBASE[g % c.NGB]
            slot2j[core, pos:pos + n] = js
            pos += int(Tdec[g]) * 128
        assert pos == TOT_DEC

    xp = np.zeros((c.NP, c.D), dtype=np.float32)
    xp[:c.N] = np.asarray(x, dtype=np.float32)
    use_b1 = bool(np.any(np.asarray(b1)))
    use_b2 = bool(np.any(np.asarray(b2)))

    in_maps = []
    for core in range(c.NC):
        sl = slice(core * c.SHARD, (core + 1) * c.SHARD)
        dsh = dinv_f[sl]
        m = {
            "xT": np.ascontiguousarray(xp[sl].T).astype(BF),
            "W1": np.asarray(W1, dtype=np.float32).astype(BF),
            "W2": np.asarray(W2, dtype=np.float32).astype(BF),
            "dinv": np.ascontiguousarray(dsh.reshape(c.NW, 128).T),
            "gidx": _wrap_idxs(idx_arr[core]),
            "dstloc": np.ascontiguousarray(
                dloc_arr[core].reshape(TOT_TILES, 128).T),
            "negdst": np.ascontiguousarray(
                (-dloc_arr[core]).reshape(TOT_TILES, 128).T),
            "didx_s": _wrap_idxs(idx_s[core]),
            "didx_d": _wrap_idxs(idx_d[core]),
        }
        if use_b1 or use_b2:
            m["b1r"] = np.asarray(b1, np.float32)[None, :].astype(BF)
            m["b2r"] = np.asarray(b2, np.float32)[None, :].astype(BF)
            m["invd"] = (1.0 / dsh)[None, :].astype(BF)
        in_maps.append(m)
    meta = dict(T=T, span_tiles=span_tiles, TOT=TOT, TOT_TILES=TOT_TILES,
                Tdec=Tdec, gorder=gorder, TOT_DEC=TOT_DEC, slot2j=slot2j,
                use_b1=use_b1, use_b2=use_b2)
    return in_maps, meta


def build_program(cfg, meta, num_cores=None):
    c = cfg
    NCores = num_cores or c.NC
    T, span_tiles = meta["T"], meta["span_tiles"]
    TOT, TOT_TILES = meta["TOT"], meta["TOT_TILES"]
    Tdec, gorder, TOT_DEC = meta["Tdec"], meta["gorder"], meta["TOT_DEC"]
    use_b = {1: meta["use_b1"], 2: meta["use_b2"]}
    D = c.D
    TBMAX = int(span_tiles.sum(axis=1).max())
    # gather band g reads exactly collective chunk g (1:1)
    GB_CHUNKS = [[g] for g in range(c.NGB)]

    nc = bacc.Bacc("TRN2", target_bir_lowering=False, debug=False,
                   num_devices=NCores, num_swdge_queues=4)
    NQ = 4

    xT_in = nc.dram_tensor("xT", [D, c.SHARD], BF16, kind="ExternalInput")
    W1_in = nc.dram_tensor("W1", [D, D], BF16, kind="ExternalInput")
    W2_in = nc.dram_tensor("W2", [D, D], BF16, kind="ExternalInput")
    dinv_in = nc.dram_tensor("dinv", [128, c.NW], F32, kind="ExternalInput")
    gidx_in = nc.dram_tensor("gidx", [128, TOT // 16], I16, kind="ExternalInput")
    dstloc_in = nc.dram_tensor("dstloc", [128, TOT_TILES], F32, kind="ExternalInput")
    negdst_in = nc.dram_tensor("negdst", [128, TOT_TILES], F32, kind="ExternalInput")
    didx_s_in = nc.dram_tensor("didx_s", [128, TOT_DEC // 16], I16, kind="ExternalInput")
    didx_d_in = nc.dram_tensor("didx_d", [128, TOT_DEC // 16], I16, kind="ExternalInput")
    if use_b[1] or use_b[2]:
        b1_in = nc.dram_tensor("b1r", [1, D], BF16, kind="ExternalInput")
        b2_in = nc.dram_tensor("b2r", [1, D], BF16, kind="ExternalInput")
        invd_in = nc.dram_tensor("invd", [1, c.SHARD], BF16, kind="ExternalInput")
    dots_out = nc.dram_tensor("dots", [128, TOT_DEC // 128], F32, kind="ExternalOutput")

    shard_b = {l: [nc.dram_tensor(f"shard{l}_{k}", [c.CH_SIZES[k], D], BF16)
                   for k in range(c.NB)] for l in (1, 2, 3)}
    table = {l: nc.dram_tensor(f"table{l}", [c.TB_STARTS[-1], D], BF16,
                               addr_space="Shared") for l in (1, 2, 3)}

    def tslice(l, g):
        return table[l][c.TB_STARTS[g]:c.TB_STARTS[g + 1], :]

    iota_dram = nc.inline_tensor(
        np.tile(np.arange(128, dtype=np.float32), (128, 1)).astype(BF), "iota_c")
    ident_dram = nc.inline_tensor(np.eye(128, dtype=np.float32).astype(BF), "ident_c")

    core_ids = list(range(NCores))
    gst = {"count": 0, "prev": None}
    ccst = {}                            # (l, K) -> collective instruction

    def emit_gather(out_ap, in_ap, idx_ap, n_idx, deps=()):
        q = gst["count"] % NQ
        inst = nc.gpsimd.dma_gather(out_ap, in_ap, idx_ap, n_idx, n_idx, D,
                                    queue_num=q, single_packet=False)
        if gst["prev"] is not None:
            add_dep_helper(inst.ins, gst["prev"].ins, sync=False,
                           reason="pin swdge queue order")
        for dcc in deps:
            add_dep_helper(inst.ins, dcc.ins, sync=True,
                           reason="gather after collective")
        gst["prev"] = inst
        gst["count"] += 1
        return inst

    def emit_collective(l, K):
        cc = nc.gpsimd.collective_compute(
            "AllGather", mybir.AluOpType.bypass,
            replica_groups=[core_ids],
            ins=[shard_b[l][K][:]],
            outs=[table[l][c.TB_STARTS[K]:c.TB_STARTS[K + 1], :]],
        )
        if gst["prev"] is not None:
            add_dep_helper(cc.ins, gst["prev"].ins, sync=False,
                           reason="order on gpsimd")
        gst["prev"] = cc
        ccst[(l, K)] = cc
        return cc

    def write_window(l, w, src_ap, done_k):
        """DMA window w rows into its chunk shard; fire collectives when a
        chunk completes (chunks are window-aligned)."""
        lo = w * 128
        K = 0
        while c.CH_STARTS[K + 1] <= lo:
            K += 1
        off = lo - c.CH_STARTS[K]
        nc.sync.dma_start(shard_b[l][K][off:off + 128, :], src_ap)
        while len(done_k) < c.NB and \
                (w + 1) * 128 >= c.CH_STARTS[len(done_k) + 1]:
            emit_collective(l, len(done_k))
            done_k.append(len(done_k))

    with tile.TileContext(nc) as tc:
        with contextlib.ExitStack() as es:
            const = es.enter_context(tc.tile_pool(name="const", bufs=1))
            meta_p = es.enter_context(tc.tile_pool(name="meta", bufs=1))

            w1_sb = const.tile([D, D], BF16); nc.sync.dma_start(w1_sb[:], W1_in[:])
            w2_sb = const.tile([D, D], BF16); nc.sync.dma_start(w2_sb[:], W2_in[:])
            dinv_sb = const.tile([128, c.NW], F32)
            nc.sync.dma_start(dinv_sb[:], dinv_in[:])
            iota_sb = const.tile([128, 128], BF16)
            nc.sync.dma_start(iota_sb[:], iota_dram[:])
            ident_sb = const.tile([128, 128], BF16)
            nc.sync.dma_start(ident_sb[:], ident_dram[:])
            if use_b[1] or use_b[2]:
                b1_sb = const.tile([1, D], BF16)
                nc.sync.dma_start(b1_sb[:], b1_in[:])
                b2_sb = const.tile([1, D], BF16)
                nc.sync.dma_start(b2_sb[:], b2_in[:])
                invd_sb = const.tile([1, c.SHARD], BF16)
                nc.sync.dma_start(invd_sb[:], invd_in[:])
            gidx_sb = meta_p.tile([128, TOT // 16], I16)
            nc.scalar.dma_start(gidx_sb[:], gidx_in[:])
            dstloc_sb = meta_p.tile([128, TOT_TILES], F32)
            nc.scalar.dma_start(dstloc_sb[:], dstloc_in[:])
            negdst_sb = meta_p.tile([128, TOT_TILES], F32)
            nc.scalar.dma_start(negdst_sb[:], negdst_in[:])

            span_base = {}
            tcol0 = 0
            for b in range(c.NBATCH):
                m0 = 0
                for g in range(c.NGB):
                    span_base[(b, g)] = (tcol0, m0)
                    tcol0 += int(span_tiles[b, g])
                    m0 += int(span_tiles[b, g])

            def emit_S(Sp, Ap, tcol):
                S = Sp.tile([128, 128], BF16, tag="S")
                if (tcol % 10) < c.ACT_TENTHS:
                    a = Ap.tile([128, 128], BF16, tag="a")
                    nc.scalar.activation(
                        a[:], iota_sb[:], ACTF.Abs,
                        bias=negdst_sb[:, tcol:tcol + 1])
                    nc.scalar.activation(
                        S[:], a[:], ACTF.Relu, scale=-1.0, bias=1.0)
                else:
                    nc.vector.tensor_scalar(
                        S[:], iota_sb[:], dstloc_sb[:, tcol:tcol + 1], None,
                        mybir.AluOpType.is_equal)
                return S

            pre = {}        # (lid, b) -> (Mt, set of emitted gbands)
            cc_waited = {1: set(), 2: set(), 3: set()}

            def batch_gathers(Mp, lid, b, only_avail=False):
                """Emit (remaining) gathers for batch b of layer lid."""
                Mt, done = pre.get((lid, b), (None, set()))
                if Mt is None:
                    Mt = Mp.tile([128, TBMAX, 128], BF16, tag="M")
                for g in range(c.NGB):
                    if g in done or int(span_tiles[b, g]) == 0:
                        continue
                    if only_avail and any((lid, K) not in ccst
                                          for K in GB_CHUNKS[g]):
                        continue
                    ntiles = int(span_tiles[b, g])
                    tb, mb = span_base[(b, g)]
                    deps = []
                    if g not in cc_waited[lid]:
                        deps = [ccst[(lid, K)] for K in GB_CHUNKS[g]]
                        cc_waited[lid].add(g)
                    emit_gather(
                        Mt[:, mb:mb + ntiles, :], tslice(lid, g),
                        gidx_sb[:, tb * 8:(tb + ntiles) * 8],
                        ntiles * 128, deps=deps)
                    done.add(g)
                pre[(lid, b)] = (Mt, done)
                return Mt

            def layer(Mp, lid, hall, hall_next, make_next, next_lid):
                waited_done = []
                with tc.tile_pool(name=f"S{lid}", bufs=32) as Sp, \
                     tc.tile_pool(name=f"A{lid}", bufs=8) as Ap, \
                     tc.tile_pool(name=f"ag{lid}", bufs=4, space="PSUM") as agp, \
                     tc.tile_pool(name=f"tp{lid}", bufs=2, space="PSUM") as tpp, \
                     tc.tile_pool(name=f"ep{lid}", bufs=4) as epp:
                    for b in range(c.NBATCH):
                        wlo, whi = b * c.WB, min((b + 1) * c.WB, c.NW)
                        Mt = batch_gathers(Mp, lid, b)
                        for w in range(wlo, whi):
                            ps = agp.tile([128, D], F32, tag="agg")
                            nmm = int(T[w].sum())
                            hsl = hall[:, w * 128:w * 128 + D]
                            nc.tensor.matmul(ps[:], lhsT=ident_sb[:],
                                             rhs=hsl,
                                             start=True, stop=(nmm == 0 and
                                                               not use_b[lid]))
                            mi = 0
                            for g in range(c.NGB):
                                tb, mb = span_base[(b, g)]
                                off = int(T[wlo:w, g].sum())
                                for t in range(int(T[w, g])):
                                    tcol = tb + off + t
                                    mcol = mb + off + t
                                    S = emit_S(Sp, Ap, tcol)
                                    mi += 1
                                    nc.tensor.matmul(
                                        ps[:], lhsT=S[:], rhs=Mt[:, mcol, :],
                                        start=False,
                                        stop=(mi == nmm and not use_b[lid]))
                            if use_b[lid]:
                                bsb = b1_sb if lid == 1 else b2_sb
                                nc.tensor.matmul(
                                    ps[:],
                                    lhsT=invd_sb[0:1, w * 128:(w + 1) * 128],
                                    rhs=bsb[:],
                                    start=False, stop=True)
                            if make_next:
                                z = epp.tile([128, D], BF16, tag="z")
                                nc.scalar.activation(
                                    z[:], ps[:], ACTF.Relu,
                                    scale=dinv_sb[:, w:w + 1])
                                zt_ps = tpp.tile([128, D], BF16, tag="zt")
                                nc.tensor.transpose(zt_ps[:], z[:], ident_sb[:])
                                zT = epp.tile([128, D], BF16, tag="zT")
                                nc.scalar.activation(zT[:], zt_ps[:], ACTF.Copy)
                                h2ps = tpp.tile([128, D], F32, tag="h2")
                                nc.tensor.matmul(h2ps[:], lhsT=zT[:],
                                                 rhs=w2_sb[:],
                                                 start=True, stop=True)
                                nc.scalar.activation(
                                    hall_next[:, w * 128:w * 128 + D], h2ps[:],
                                    ACTF.Copy, scale=dinv_sb[:, w:w + 1])
                            else:
                                nc.scalar.activation(
                                    hall_next[:, w * 128:w * 128 + D], ps[:],
                                    ACTF.Relu, scale=dinv_sb[:, w:w + 1])
                            write_window(next_lid, w,
                                         hall_next[:, w * 128:w * 128 + D],
                                         waited_done)
                        if b == c.NBATCH - 1 - c.PREFETCH and next_lid == 2:
                            for pb in range(min(c.PREFETCH + 1, c.NBATCH)):
                                batch_gathers(Mp, 2, pb, only_avail=True)

            with tc.tile_pool(name="hs", bufs=1) as hsp, \
                 tc.tile_pool(name="Mpool", bufs=3 + c.PREFETCH) as Mp:
                hs1_all = hsp.tile([128, c.SHARD], BF16, tag="hs1")
                hs2_all = hsp.tile([128, c.SHARD], BF16, tag="hs2")
                z_all = hsp.tile([128, c.SHARD], BF16, tag="z")
                # P0: hs1 = dinv * (x @ W1), xT resident
                done0 = []
                with tc.tile_pool(name="p0x", bufs=1) as p0x, \
                     tc.tile_pool(name="p0ps", bufs=4, space="PSUM") as p0ps:
                    xT_sb = p0x.tile([D, c.SHARD], BF16)
                    for K in range(c.NB):
                        lo, hi = c.CH_STARTS[K], c.CH_STARTS[K + 1]
                        nc.sync.dma_start(xT_sb[:, lo:hi], xT_in[:, lo:hi])
                    for w in range(c.NW):
                        ps = p0ps.tile([128, D], F32, tag="ps")
                        nc.tensor.matmul(
                            ps[:], lhsT=xT_sb[:, w * 128:(w + 1) * 128],
                            rhs=w1_sb[:], start=True, stop=True)
                        nc.scalar.activation(
                            hs1_all[:, w * 128:w * 128 + D], ps[:],
                            ACTF.Copy, scale=dinv_sb[:, w:w + 1])
                        write_window(1, w, hs1_all[:, w * 128:w * 128 + D],
                                     done0)
                layer(Mp, 1, hs1_all, hs2_all, True, 2)
                layer(Mp, 2, hs2_all, z_all, False, 3)

            # decode
            with tc.tile_pool(name="didx", bufs=1) as didxp, \
                 tc.tile_pool(name="dM", bufs=1) as dMp, \
                 tc.tile_pool(name="dw", bufs=6) as dwp, \
                 tc.tile_pool(name="dout", bufs=1) as doutp:
                ds_sb = didxp.tile([128, TOT_DEC // 16], I16)
                nc.scalar.dma_start(ds_sb[:], didx_s_in[:])
                dd_sb = didxp.tile([128, TOT_DEC // 16], I16)
                nc.scalar.dma_start(dd_sb[:], didx_d_in[:])
                Ms = dMp.tile([128, TOT_DEC // 128, D], BF16, tag="Ms")
                Md = dMp.tile([128, TOT_DEC // 128, D], BF16, tag="Md")
                res = doutp.tile([128, TOT_DEC // 128], F32)
                waited = set()
                coff = 0
                for g in gorder:
                    ks, kd = g // c.NGB, g % c.NGB
                    ncols = int(Tdec[g])
                    if ncols == 0:
                        continue
                    dep_s, dep_d = [], []
                    if ks not in waited:
                        dep_s = [ccst[(3, K)] for K in GB_CHUNKS[ks]]
                        waited.add(ks)
                    if kd not in waited:
                        dep_d = [ccst[(3, K)] for K in GB_CHUNKS[kd]]
                        waited.add(kd)
                    off16 = coff * 8
                    emit_gather(Ms[:, coff:coff + ncols, :], tslice(3, ks),
                                ds_sb[:, off16:off16 + ncols * 8], ncols * 128,
                                deps=dep_s)
                    emit_gather(Md[:, coff:coff + ncols, :], tslice(3, kd),
                                dd_sb[:, off16:off16 + ncols * 8], ncols * 128,
                                deps=dep_d)
                    for t in range(ncols):
                        col = coff + t
                        mm = dwp.tile([128, D], F32, tag="mm")
                        nc.vector.tensor_tensor(
                            mm[:], Ms[:, col, :], Md[:, col, :],
                            op=mybir.AluOpType.mult)
                        trash = dwp.tile([128, D], BF16, tag="tr")
                        nc.scalar.activation(
                            trash[:], mm[:], ACTF.Copy,
                            accum_out=res[:, col:col + 1])
                    coff += ncols
                nc.sync.dma_start(dots_out[:], res[:])

    nc.compile()
    return nc


def assemble_output(cfg, meta, results):
    c = cfg
    slot2j = meta["slot2j"]
    out = np.zeros(c.EL, dtype=np.float32)
    for core in range(len(results)):
        d = np.asarray(results[core]["dots"], dtype=np.float32)
        flat = d.T.reshape(-1)             # slot i -> d[i%128, i//128]
        s2j = slot2j[core]
        valid = s2j >= 0
        out[s2j[valid]] = flat[valid]
    return out


def run_pipeline(x, edge_index, edge_label_index, W1, b1, W2, b2,
                 cfg=None, trace=False, tmpdir=None):
    cfg = cfg or DEFAULT
    in_maps, meta = host_prep(cfg, x, edge_index, edge_label_index,
                              W1, b1, W2, b2)
    nc = build_program(cfg, meta)
    res = run_bass_kernel_spmd(nc, in_maps, list(range(cfg.NC)),
                               trace=trace, tmpdir=tmpdir)
    return assemble_output(cfg, meta, res.results), res


def kernel(x, edge_index, edge_label_index, W1, b1, W2, b2):
    out, _ = run_pipeline(x, edge_index, edge_label_index, W1, b1, W2, b2)
    return out
